# revision 25
# baseline (speedup 1.0000x reference)
"""Trainium2 Bass kernel for nn_BestChangeLayer (GoL pattern search).

Math: for each batch b, the 7x7 window W of x at (ry,rx) gets its center 3x3
replaced by each of 512 patterns p; one Game-of-Life step runs and the inner
5x5 is compared with the target window tw. Since GoL is new = [s==3]+[s==2]*c
(s = 8-neighbor sum) and |new-tw| = w*new + tw with w = 1-2*tw, the
512-pattern sweep collapses to errors = G^T@T + const(b). G packs BOTH
feature families into exactly 128 rows (cells m colmajor over the 5x5):
  - A-rows: w(m) * [S_fix(m) == vA], vA in {3,2,1,0}  (84 rows whose
    pattern-side table row is nonzero)
  - B-rows: w(m)*c(m)*[S_fix(m) == vB], vB in {2,1,0} (ring cells; 44 rows)
  - rows whose table row is identically zero (cells that cannot reach the
    required S_pat) are pruned: 84 + 44 = 128 exactly.
  - const(b) = sum(tw) is pattern-independent, so it cannot change the
    argmin and is dropped entirely.
S_fix / the center value c come from the fixed 7x7 context; the pattern side
([S_pat==...] indicators) is folded into the constant table T. All values
are small integers: exact in fp8(e4m3)/fp32.

Device pipeline (batch B=128 in partitions for the argmin side, window
features in partitions for the matmul side):
  - host packs wtm (128, 420) fp8 = [tT(packed rows) | stageWT | M | mskv]
    where stageWT rows 0..48 = W window colmajor + ones row, tT row r = the
    t value at row r's cell (pure gather/layout - no host arithmetic).
  - S = M^T @ stageWT: S[r] = S_fix - vA (A-rows) or
    S_fix - 16*c + 16 - vB (B-rows; zero iff c==1 and S_fix==vB);
    compare constants folded via the ones row.
  - WA = 1 - 2*tT (one DVE op); G = [S==0]*WA (one DVE op)
  - E_psum = G^T@(-T) + (-0.5 I)^T @ noise_f32r = -(errors + 0.5*noise)
    + const  (noise rounds to f32r ~2^-14; measured zero argmin flips at
    even 2^-10 rounding). A filler matmul gated on G keeps the PE clock
    ramping so the E matmul runs at full p-state while absorbing the
    table-DMA wait.
  - Max + MaxIndex (first index) straight on PSUM; bits = (idx & mskv) != 0
    written into the x passthrough tile; row stores.

DMA plan (HWDGE ring serializes ~625ns dispatch; transfers serialize on the
DMA engines, so order = readiness order): SP ring: wtm, ptab8, x, final
store; Pool SWDGE: noise (plus the f32r identity build on the Pool engine);
Act queue: the 2 early passthrough row stores. The noise->f32r rounding
copy runs on the DVE in its idle window (walrus requires an engine producer
for f32r matmul inputs; f32r rounding is argmin-safe, see above).

Sharding: pure data parallel, batch 1024 = 8 cores x 128 rows.
"""

import os
import sys

import numpy as np

for _p in ("/opt/trn_rl_repo", "/root/.axon_site/_ro/trn_rl_repo"):
    if os.path.isdir(_p) and _p not in sys.path:
        sys.path.insert(0, _p)

import ml_dtypes  # noqa: E402

import concourse.bass as bass  # noqa: E402,F401
import concourse.mybir as mybir  # noqa: E402
import concourse.tile as tile  # noqa: E402
from concourse import bacc  # noqa: E402
from concourse.bass_utils import run_bass_kernel_spmd  # noqa: E402

N_CORES = 8
B_TOTAL = 1024
B = B_TOTAL // N_CORES  # 128 batch rows per core
H = W = 25
NPAT = 512

VA = [3, 2, 1, 0]  # Ga slot k: [S_fix == VA[k]]
VB = [2, 1, 0]     # Gb slot k: c * [S_fix == VB[k]] (ring cells)

F32 = mybir.dt.float32
F32R = mybir.dt.float32r
BF16 = mybir.dt.bfloat16
FP8 = mybir.dt.float8e4
U32 = mybir.dt.uint32

F8NP = ml_dtypes.float8_e4m3

# wtm layout (128 rows, 420 fp8 cols):
# [tT(packed) 0:128 | stageWT(50 rows) 128:256 | M 256:384 | mskv 384:420]
WTM_ROWS = 128
WTM_COLS = 420


# ---------------------------------------------------------------------------
# Host-side constants. Cells indexed colmajor: m = c5*5 + r5 for the output
# cell (r5, c5); 7x7 window positions (u,v) map to stageW row v*7+u.
# ---------------------------------------------------------------------------
def _cell_geom():
    geom = []
    for m in range(25):
        r5, c5 = m % 5, m // 5
        r, c = r5 + 1, c5 + 1  # 7x7 coords
        nb_fix, nb_pat = [], []
        for dr in (-1, 0, 1):
            for dc in (-1, 0, 1):
                if dr == 0 and dc == 0:
                    continue
                u, v = r + dr, c + dc
                (nb_pat if (2 <= u <= 4 and 2 <= v <= 4) else nb_fix).append((u, v))
        geom.append((nb_fix, nb_pat, (r, c), 2 <= r <= 4 and 2 <= c <= 4))
    return geom


def _packed_rows():
    """The 128 (kind, k, m) rows whose pattern-side table row is nonzero:
    84 A-rows (w*[S_fix==VA[k]]) + 44 B-rows (w*c*[S_fix==VB[k]], ring).
    Exactly 128 -- Ga/Gb pack into a single G."""
    geom = _cell_geom()
    ints = np.arange(NPAT)
    shifts = np.arange(8, -1, -1)
    pats = ((ints[:, None] >> shifts[None, :]) & 1).astype(np.float32).reshape(NPAT, 3, 3)
    S_pat = np.zeros((NPAT, 25), np.float32)
    C_pat = np.zeros((NPAT, 25), np.float32)
    for m, (nb_fix, nb_pat, (r, c), inner) in enumerate(geom):
        for (u, v) in nb_pat:
            S_pat[:, m] += pats[:, u - 2, v - 2]
        if inner:
            C_pat[:, m] = pats[:, r - 2, c - 2]
    rows = []
    for k, vA in enumerate(VA):
        for m, (_, _, _, inner) in enumerate(geom):
            t1 = (S_pat[:, m] == 3 - vA).astype(np.float32)
            if inner:
                t1 = t1 + C_pat[:, m] * (S_pat[:, m] == 2 - vA)
            if t1.any():
                rows.append(("A", k, m, t1))
    for k, vB in enumerate(VB):
        for m, (_, _, _, inner) in enumerate(geom):
            if not inner:
                t1 = (S_pat[:, m] == 2 - vB).astype(np.float32)
                if t1.any():
                    rows.append(("B", k, m, t1))
    assert len(rows) == 128, len(rows)
    return rows


def _build_tables():
    """Returns (mconst (128, 164) fp8 = [M|mskv], ptab8 (128, 512) fp8)."""
    geom = _cell_geom()
    rows = _packed_rows()

    M = np.zeros((128, 128), np.float32)
    T = np.zeros((128, NPAT), np.float32)
    for r, (kind, k, m, t1) in enumerate(rows):
        nb_fix, _, (rr, cc), inner = geom[m]
        for (u, v) in nb_fix:
            M[v * 7 + u, r] += 1.0
        if kind == "A":
            M[49, r] = -float(VA[k])
        else:
            M[cc * 7 + rr, r] = -16.0
            M[49, r] = 16.0 - float(VB[k])
        T[r] = t1

    mskv = (1 << np.arange(8, -1, -1, dtype=np.uint32))[None, :].repeat(128, 0)
    mskv_f8 = np.ascontiguousarray(mskv).view(np.uint8).view(F8NP)

    mconst = np.zeros((128, 164), F8NP)
    mconst[0:50, 0:128] = M[0:50].astype(F8NP)
    mconst[:, 128:164] = mskv_f8
    assert np.array_equal(np.asarray(mconst[0:50, 0:128], np.float32), M[0:50])

    ptab8 = (-T).astype(F8NP)
    assert np.array_equal(np.asarray(ptab8, np.float32), -T)
    return mconst, ptab8


def _host_stage(x3, t3, ry, rx):
    """Returns (stageWT (50, Btot), tT4 (128, Btot)) fp8: stageWT rows 0..48 =
    W window colmajor (row v*7+u), row 49 = ones; tT4 = the 5x5 t window
    (colmajor) replicated into four 32-row slots. Pure gather/constant
    layout, no arithmetic."""
    Btot = x3.shape[0]
    wrows = [(ry - 2 + i) % H for i in range(7)]
    wcols = [(rx - 2 + j) % W for j in range(7)]
    trows = [(ry - 1 + i) % H for i in range(5)]
    tcols = [(rx - 1 + j) % W for j in range(5)]
    Wwin = x3[:, wrows, :][:, :, wcols]              # (Btot, 7, 7) [u, v]
    twin = t3[:, trows, :][:, :, tcols]              # (Btot, 5, 5) [r, c]
    stageWT = np.zeros((50, Btot), F8NP)
    stageWT[0:49] = Wwin.transpose(2, 1, 0).reshape(49, Btot).astype(F8NP)
    stageWT[49] = F8NP(1.0)
    t25 = twin.transpose(2, 1, 0).reshape(25, Btot).astype(F8NP)  # colmajor
    m_of_row = [m for (_, _, m, _) in _packed_rows()]
    tT4 = np.ascontiguousarray(t25[m_of_row])  # (128, Btot) packed
    return stageWT, tT4


# ---------------------------------------------------------------------------
# Kernel builder
# ---------------------------------------------------------------------------
_CACHE = {}


def _build(ry, rx):
    assert 0 <= ry <= H - 3 and 0 <= rx <= W - 3, (ry, rx)
    _, ptab8_np = _build_tables()
    OP = mybir.AluOpType

    nc = bacc.Bacc(None, target_bir_lowering=False)
    x_h = nc.dram_tensor("x", [B, H * W], F32, kind="ExternalInput")
    wtm_h = nc.dram_tensor("wtm", [WTM_ROWS, WTM_COLS], FP8, kind="ExternalInput")
    n_h = nc.dram_tensor("noise", [B, NPAT], F32, kind="ExternalInput")
    o_h = nc.dram_tensor("out", [B, H * W], F32, kind="ExternalOutput")
    ptab8_h = nc.inline_tensor(ptab8_np, "ptab8")

    with tile.TileContext(nc) as tc:
        with (
            tc.tile_pool(name="sb", bufs=1) as sb,
            tc.tile_pool(name="ps", bufs=1, space="PSUM") as ps,
        ):
            # ---- SP(sync) ring DMAs in readiness/deadline order ----
            wtm = sb.tile([WTM_ROWS, WTM_COLS], FP8)
            nc.sync.dma_start(out=wtm[:], in_=wtm_h[:, :])
            ptab8 = sb.tile([128, NPAT], FP8)
            nc.sync.dma_start(out=ptab8[:], in_=ptab8_h[:, :])
            x_tile = sb.tile([B, H * W], F32)
            nc.sync.dma_start(out=x_tile[:], in_=x_h[:, :])

            # ---- Pool queue: noise via SWDGE (bypasses the ring) ----
            noise = sb.tile([B, NPAT], F32)
            nc.gpsimd.dma_start(out=noise[:], in_=n_h[:, :])
            # -0.5 identity in f32r for the noise matmul
            identn = sb.tile([128, 128], F32R)
            nc.gpsimd.memset(identn[:].bitcast(F32), 0.0)
            nc.gpsimd.affine_select(
                out=identn[:], in_=identn[:].bitcast(F32),
                compare_op=OP.not_equal, fill=-0.5,
                base=0, pattern=[[-1, 128]], channel_multiplier=1)

            # ---- DVE: warm tile memset; WA; Ga; Gb ----
            warm_in = sb.tile([128, 128], BF16)
            nc.vector.memset(warm_in[:], 0.0)

            # ---- PE warmup (starts the clock ramp early) ----
            warm_ps = ps.tile([8, 128], F32)
            for _ in range(5):
                nc.tensor.matmul(warm_ps[:], warm_in[:, 0:8], warm_in[:],
                                 start=True, stop=True)

            # WA = 1 - 2*tT (values -1/+1; fp8-exact)
            WA = sb.tile([128, B], FP8)
            nc.vector.tensor_scalar(WA[:], wtm[:, 0:128], -2.0, 1.0,
                                    OP.mult, OP.add)

            # single S matmul (compare constants folded via the ones row)
            S_ps = ps.tile([128, B], F32)
            nc.tensor.matmul(S_ps[:], wtm[0:50, 256:384], wtm[0:50, 128:256],
                             start=True, stop=True)

            # single packed G = [S==0]*WA
            G_sb = sb.tile([128, B], FP8)
            nc.vector.scalar_tensor_tensor(
                out=G_sb[:], in0=S_ps[:], scalar=0.0, in1=WA[:],
                op0=OP.is_equal, op1=OP.mult)

            # ---- noise -> f32r rounding copy (DVE is idle in this window;
            # walrus requires an engine producer for f32r matmul inputs) ----
            noise_r = sb.tile([B, NPAT], F32R)
            nc.vector.tensor_copy(out=noise_r[:], in_=noise[:])

            # keep the PE clock ramping right up to the E matmul (p-state)
            filler_ps = ps.tile([8, 150], F32)
            nc.tensor.matmul(filler_ps[:], G_sb[:, 0:8], wtm[:, 0:150],
                             start=True, stop=True)
            # ---- E accumulation: negseed (+ const) in PSUM ----
            E_ps = ps.tile([B, NPAT], F32)
            nc.tensor.matmul(E_ps[:], G_sb[:], ptab8[:],
                             start=True, stop=False)
            nc.tensor.matmul(E_ps[:], identn[:], noise_r[:],
                             start=False, stop=True)

            # ---- argmax: Max + MaxIndex straight from PSUM ----
            mx8 = sb.tile([B, 8], F32)
            nc.vector.max(out=mx8[:], in_=E_ps[:])
            idx8 = sb.tile([B, 8], U32)
            nc.vector.max_index(out=idx8[:], in_max=mx8[:], in_values=E_ps[:])

            # ---- bits -> patch written into x passthrough tile ----
            mskv = wtm[:, 384:420].bitcast(U32)  # (128, 9) = 1 << (8-j)
            andv = sb.tile([B, 9], U32)
            nc.vector.tensor_tensor(
                out=andv[:], in0=idx8[:, 0:1].to_broadcast([B, 9]),
                in1=mskv, op=OP.bitwise_and)
            x3v = x_tile[:].rearrange("b (h w) -> b h w", h=H)
            nc.vector.tensor_scalar(
                x3v[:, ry:ry + 3, rx:rx + 3],
                andv[:].rearrange("b (h w) -> b h w", h=3), 0, None,
                OP.not_equal)

            # ---- stores ----
            if ry > 0:
                nc.scalar.dma_start(out=o_h[:, 0:ry * W], in_=x_tile[:, 0:ry * W])
            if ry + 3 < H:
                nc.scalar.dma_start(out=o_h[:, (ry + 3) * W:],
                                    in_=x_tile[:, (ry + 3) * W:])
            nc.sync.dma_start(out=o_h[:, ry * W:(ry + 3) * W],
                              in_=x_tile[:, ry * W:(ry + 3) * W])

    nc.finalize()
    return nc


def _get(ry, rx):
    key = (ry, rx)
    if key not in _CACHE:
        _CACHE[key] = _build(ry, rx)
    return _CACHE[key]


def kernel_with_results(x, target, noise, ry, rx, trace=False):
    x = np.ascontiguousarray(np.asarray(x, dtype=np.float32))
    target = np.ascontiguousarray(np.asarray(target, dtype=np.float32))
    noise = np.ascontiguousarray(np.asarray(noise, dtype=np.float32))
    ry, rx = int(ry), int(rx)
    Btot = x.shape[0]
    assert Btot == B_TOTAL and x.shape == (Btot, 1, H, W), x.shape

    nc = _get(ry, rx)
    xs = x.reshape(Btot, H * W)
    x3 = xs.reshape(Btot, H, W)
    t3 = target.reshape(Btot, H, W)
    stageWT, tT4 = _host_stage(x3, t3, ry, rx)  # (50/128, Btot) fp8
    mconst, _ = _build_tables()                 # (128, 292) fp8

    in_maps = []
    for c in range(N_CORES):
        wtm = np.zeros((WTM_ROWS, WTM_COLS), F8NP)
        wtm[:, 0:128] = tT4[:, c * B:(c + 1) * B]
        wtm[0:50, 128:256] = stageWT[:, c * B:(c + 1) * B]
        wtm[:, 256:420] = mconst
        in_maps.append({
            "x": xs[c * B:(c + 1) * B],
            "wtm": np.ascontiguousarray(wtm),
            "noise": noise[c * B:(c + 1) * B],
        })
    res = run_bass_kernel_spmd(nc, in_maps, core_ids=list(range(N_CORES)), trace=trace)
    out = np.concatenate([res.results[c]["out"] for c in range(N_CORES)], axis=0)
    return out.reshape(Btot, 1, H, W).astype(np.float32), res


def kernel(x, target, noise, ry, rx):
    out, _ = kernel_with_results(x, target, noise, ry, rx)
    return out
